# revision 4
# baseline (speedup 1.0000x reference)
"""GCN layer (SpMM + dense + dropout/relu) on 8 Trainium2 NeuronCores.

Strategy
--------
Destination-node sharding: core c owns output rows [c*RPC, (c+1)*RPC).
Edges are partitioned by destination owner on the host, sorted by
(dest-block, source-half), padded to 128-edge chunks.

Per core, per 128-edge chunk:
  - G[128 edges, 128 dims] = X_bf16[cols]  via one batched SWDGE dma_gather
  - S[128 edges, 128 dest] = (iota == r_rel) * val   (one DVE tensor_scalar)
  - H_T[dims, dest-block] += G.T @ S                 (TensorE, PSUM accumulate)
Per 128-row dest block:
  - OUT = H @ W + b     (bias seeded by a rank-1 ones@b matmul into PSUM)
  - out = relu(OUT) * ((drop_u >= 0.5) * 2)          (2 DVE ops)

No collectives: each core gathers from its own replica of X.
dma_gather indices are int16, so X is split in two < 32768-row tables and
each (block, half) edge run is padded to a 128 multiple.
"""

import sys

for _p in ("/opt/trn_rl_repo",):
    if _p not in sys.path:
        sys.path.append(_p)

import numpy as np
import ml_dtypes
from contextlib import ExitStack

from concourse import bass, bacc, mybir, tile
from concourse import bass_utils

P = 128
NCORES = 8
BPB = 7  # dest blocks per gather batch
P_DROP = 0.5

_dt = mybir.dt
_op = mybir.AluOpType


def _preprocess(rows, cols, vals, N):
    """Sort/pad edges into per-core gather + selector streams.

    Returns dict with per-core host arrays and the (shared) program structure.
    """
    E = rows.shape[0]
    rows = np.asarray(rows, dtype=np.int64)
    cols = np.asarray(cols, dtype=np.int64)
    vals = np.asarray(vals, dtype=np.float32)

    RPC = -(-N // (NCORES * P)) * P  # rows per core, multiple of 128
    NB = RPC // P                    # dest blocks per core
    split = min(-(-N // 2), 32000)
    assert split < 32768 and (N - split) < 32768

    core = rows // RPC
    binc = (rows % RPC) // P
    r_rel = (rows % P).astype(np.float32)
    half = (cols >= split).astype(np.int64)
    idx16 = np.where(half == 1, cols - split, cols).astype(np.int16)

    group = (core * NB + binc) * 2 + half
    order = np.argsort(group, kind="stable")
    g_sorted = group[order]

    gcounts = np.bincount(group, minlength=NCORES * NB * 2)
    gstarts = np.concatenate([[0], np.cumsum(gcounts)])[:-1]
    rank = np.arange(E, dtype=np.int64) - gstarts[g_sorted]

    # chunks per (block, half): max over cores, at least 1
    counts = gcounts.reshape(NCORES, NB, 2)
    k = np.maximum(1, -(-counts.max(axis=0) // P))  # [NB, 2]

    # slot layout per core: per batch of BPB blocks: all A halves, then all B
    batches = [list(range(i, min(i + BPB, NB))) for i in range(0, NB, BPB)]
    group_order = []
    for bl in batches:
        group_order += [(b, 0) for b in bl] + [(b, 1) for b in bl]
    sizes = np.array([k[b, h] * P for (b, h) in group_order], dtype=np.int64)
    offs = np.concatenate([[0], np.cumsum(sizes)])
    T_slots = int(offs[-1])
    T_chunks = T_slots // P
    slot_off = np.zeros((NB, 2), dtype=np.int64)
    for gi, (b, h) in enumerate(group_order):
        slot_off[b, h] = offs[gi]

    bh_sorted = g_sorted % (NB * 2)
    c_sorted = g_sorted // (NB * 2)
    slot = slot_off.reshape(-1)[bh_sorted] + rank
    assert (rank < k.reshape(-1)[bh_sorted] * P).all()

    idx_pad = np.zeros((NCORES, T_slots), np.int16)
    v_pad = np.zeros((NCORES, T_slots), np.float32)
    r_pad = np.zeros((NCORES, T_slots), np.float32)
    idx_pad[c_sorted, slot] = idx16[order]
    v_pad[c_sorted, slot] = vals[order]
    r_pad[c_sorted, slot] = r_rel[order]

    # idx element k lives at [k%16, k//16]; the Q7 SWDGE cores each read
    # their own 16-partition group, so replicate into all 8 groups.
    idx_w = np.zeros((NCORES, 128, T_slots // 16), np.int16)
    wrapped = idx_pad.reshape(NCORES, T_slots // 16, 16).transpose(0, 2, 1)
    for g in range(8):
        idx_w[:, g * 16:(g + 1) * 16, :] = wrapped
    r_w = np.ascontiguousarray(r_pad.reshape(NCORES, T_chunks, P).transpose(0, 2, 1))
    v_w = np.ascontiguousarray(v_pad.reshape(NCORES, T_chunks, P).transpose(0, 2, 1))

    return dict(
        RPC=RPC, NB=NB, split=split, k=k, batches=batches,
        slot_off=slot_off, T_slots=T_slots, T_chunks=T_chunks,
        idx_w=idx_w, r_w=r_w, v_w=v_w,
    )


def _build(N, meta):
    """Build the (per-core identical) Tile program."""
    NB = meta["NB"]
    RPC = meta["RPC"]
    split = meta["split"]
    k = meta["k"]
    batches = meta["batches"]
    slot_off = meta["slot_off"]
    T_slots = meta["T_slots"]
    T_chunks = meta["T_chunks"]

    nc = bacc.Bacc("TRN2", target_bir_lowering=False, debug=False)
    xb = nc.dram_tensor("xb", [N, P], _dt.bfloat16, kind="ExternalInput").ap()
    wt = nc.dram_tensor("wt", [P, P], _dt.bfloat16, kind="ExternalInput").ap()
    bt = nc.dram_tensor("bt", [1, P], _dt.bfloat16, kind="ExternalInput").ap()
    io = nc.dram_tensor("io", [P, P], _dt.bfloat16, kind="ExternalInput").ap()
    on = nc.dram_tensor("on", [1, P], _dt.bfloat16, kind="ExternalInput").ap()
    ix = nc.dram_tensor("ix", [128, T_slots // 16], _dt.int16, kind="ExternalInput").ap()
    rr = nc.dram_tensor("rr", [128, T_chunks], _dt.float32, kind="ExternalInput").ap()
    vv = nc.dram_tensor("vv", [128, T_chunks], _dt.float32, kind="ExternalInput").ap()
    du = nc.dram_tensor("du", [RPC, P], _dt.float32, kind="ExternalInput").ap()
    out = nc.dram_tensor("out", [RPC, P], _dt.float32, kind="ExternalOutput").ap()

    with tile.TileContext(nc) as tc, ExitStack() as ctx:
        const = ctx.enter_context(tc.tile_pool(name="const", bufs=1))
        g_pool = ctx.enter_context(tc.tile_pool(name="g", bufs=2))
        ix_pool = ctx.enter_context(tc.tile_pool(name="ix", bufs=2))
        rv_pool = ctx.enter_context(tc.tile_pool(name="rv", bufs=2))
        s_pool = ctx.enter_context(tc.tile_pool(name="s", bufs=6))
        h_pool = ctx.enter_context(tc.tile_pool(name="h", bufs=3))
        du_pool = ctx.enter_context(tc.tile_pool(name="du", bufs=3))
        ep_pool = ctx.enter_context(tc.tile_pool(name="ep", bufs=6))
        psum_h = ctx.enter_context(tc.tile_pool(name="ph", bufs=2, space="PSUM"))
        psum_o = ctx.enter_context(tc.tile_pool(name="po", bufs=2, space="PSUM"))

        iota_t = const.tile([P, P], _dt.bfloat16)
        nc.sync.dma_start(iota_t[:], io)
        w_t = const.tile([P, P], _dt.bfloat16)
        nc.sync.dma_start(w_t[:], wt)
        b_t = const.tile([1, P], _dt.bfloat16)
        nc.sync.dma_start(b_t[:], bt)
        ones_t = const.tile([1, P], _dt.bfloat16)
        nc.sync.dma_start(ones_t[:], on)

        for blocks in batches:
            cA = int(sum(k[b, 0] for b in blocks))
            cB = int(sum(k[b, 1] for b in blocks))
            ctot = cA + cB
            base_chunk = int(slot_off[blocks[0], 0]) // P
            base_slot = base_chunk * P

            G = g_pool.tile([P, ctot * P], _dt.bfloat16, tag="G")
            ixA = ix_pool.tile([128, cA * 8], _dt.int16, tag="ixA")
            nc.sync.dma_start(
                ixA[:], ix[:, base_slot // 16: base_slot // 16 + cA * 8])
            ixB = ix_pool.tile([128, cB * 8], _dt.int16, tag="ixB")
            bslotB = base_slot + cA * P
            nc.sync.dma_start(
                ixB[:], ix[:, bslotB // 16: bslotB // 16 + cB * 8])
            rt = rv_pool.tile([128, ctot], _dt.float32, tag="rt")
            nc.sync.dma_start(rt[:], rr[:, base_chunk: base_chunk + ctot])
            vt = rv_pool.tile([128, ctot], _dt.float32, tag="vt")
            nc.sync.dma_start(vt[:], vv[:, base_chunk: base_chunk + ctot])

            gA = G[:, 0: cA * P].rearrange("p (c e) -> p c e", e=P)
            nc.gpsimd.dma_gather(
                out_ap=gA, in_ap=xb[0:split, :], idxs_ap=ixA[:],
                num_idxs=cA * P, num_idxs_reg=cA * P, elem_size=P,
                single_packet=False)
            gB = G[:, cA * P: ctot * P].rearrange("p (c e) -> p c e", e=P)
            nc.gpsimd.dma_gather(
                out_ap=gB, in_ap=xb[split:N, :], idxs_ap=ixB[:],
                num_idxs=cB * P, num_idxs_reg=cB * P, elem_size=P,
                single_packet=False)

            for b in blocks:
                chunks = []
                for h in (0, 1):
                    c0 = int(slot_off[b, h]) // P - base_chunk
                    chunks += list(range(c0, c0 + int(k[b, h])))
                Hp = psum_h.tile([P, P], _dt.float32)
                for i, lc in enumerate(chunks):
                    S = s_pool.tile([P, P], _dt.bfloat16, tag="S")
                    nc.vector.tensor_scalar(
                        out=S[:], in0=iota_t[:],
                        scalar1=rt[:, lc: lc + 1], scalar2=vt[:, lc: lc + 1],
                        op0=_op.is_equal, op1=_op.mult)
                    nc.tensor.matmul(
                        out=Hp[:], lhsT=G[:, lc * P: (lc + 1) * P], rhs=S[:],
                        start=(i == 0), stop=(i == len(chunks) - 1))
                Hs = h_pool.tile([P, P], _dt.bfloat16, tag="Hs")
                nc.scalar.copy(Hs[:], Hp[:])
                Op = psum_o.tile([P, P], _dt.float32)
                nc.tensor.matmul(Op[:], lhsT=ones_t[:], rhs=b_t[:],
                                 start=True, stop=False)
                nc.tensor.matmul(Op[:], lhsT=Hs[:], rhs=w_t[:],
                                 start=False, stop=True)
                dut = du_pool.tile([P, P], _dt.float32, tag="du")
                nc.sync.dma_start(dut[:], du[b * P: (b + 1) * P, :])
                m2 = ep_pool.tile([P, P], _dt.float32, tag="m2")
                nc.vector.tensor_scalar(
                    out=m2[:], in0=dut[:], scalar1=float(P_DROP),
                    scalar2=1.0 / (1.0 - P_DROP),
                    op0=_op.is_ge, op1=_op.mult)
                ot = ep_pool.tile([P, P], _dt.float32, tag="ot")
                nc.vector.scalar_tensor_tensor(
                    out=ot[:], in0=Op[:], scalar=0.0, in1=m2[:],
                    op0=_op.max, op1=_op.mult)
                nc.sync.dma_start(out[b * P: (b + 1) * P, :], ot[:])

    nc.compile()
    return nc


def _make_in_maps(X, W, b, drop_u, meta):
    N = X.shape[0]
    RPC = meta["RPC"]
    bf = ml_dtypes.bfloat16
    xb = np.ascontiguousarray(X.astype(bf))
    wt = np.ascontiguousarray(W.astype(bf))
    bt = np.ascontiguousarray(b.reshape(1, P).astype(bf))
    io = np.ascontiguousarray(
        np.broadcast_to(np.arange(P, dtype=np.float32), (P, P)).astype(bf))
    on = np.ones((1, P), dtype=bf)
    du_pad = np.ones((NCORES * RPC, P), np.float32)
    du_pad[:N] = drop_u
    in_maps = []
    for c in range(NCORES):
        in_maps.append(dict(
            xb=xb, wt=wt, bt=bt, io=io, on=on,
            ix=meta["idx_w"][c], rr=meta["r_w"][c], vv=meta["v_w"][c],
            du=np.ascontiguousarray(du_pad[c * RPC: (c + 1) * RPC]),
        ))
    return in_maps


def kernel(rows, cols, vals, X, W, b, drop_u):
    N = X.shape[0]
    assert X.shape[1] == P and W.shape == (P, P)
    meta = _preprocess(rows, cols, vals, N)
    nc = _build(N, meta)
    in_maps = _make_in_maps(
        np.asarray(X, np.float32), np.asarray(W, np.float32),
        np.asarray(b, np.float32), np.asarray(drop_u, np.float32), meta)
    res = bass_utils.run_bass_kernel_spmd(
        nc, in_maps, core_ids=list(range(NCORES)))
    out = np.concatenate([res.results[c]["out"] for c in range(NCORES)], axis=0)
    return out[:N].astype(np.float32)


# revision 6
# speedup vs baseline: 1.6849x; 1.6849x over previous
"""GCN layer (SpMM + dense + dropout/relu) on 8 Trainium2 NeuronCores.

Strategy (v2)
-------------
Destination-node sharding: core c owns output rows [c*RPC, (c+1)*RPC).
Edges are partitioned by destination owner on the host, sorted by
(dest-block, source-half), padded to 128-edge chunks.

Per core, per 128-edge chunk:
  - G[128 edges, 128 dims] = X_bf16[cols] via batched SWDGE dma_gather,
    round-robined over all 4 SWDGE queues (4 Q7 core-pairs generate
    descriptors in parallel; ~2.7 ns/edge wall).
  - S[128 edges, 128 dest] = vals * onehot(dest_rel): HOST-built bf16
    stream, DMA'd in per batch (kills the per-chunk DVE one-hot build,
    which dominates when built on-device: ~600ns fixed per DVE op).
  - H_T[dims, dest-block] += G.T @ S   (TensorE, PSUM accumulate)
Per 128-row dest block:
  - OUT = H @ W + b   (bias seeded by a rank-1 ones@b matmul into PSUM)
  - out = relu(OUT) * ((drop_u >= 0.5) * 2)   (2 DVE ops)
drop_u loads and out stores are batched per 7-block gather batch.

No collectives: each core gathers from its own replica of X.
dma_gather indices are int16, so X is split in two < 32768-row tables and
each (block, half) edge run is padded to a 128 multiple.
"""

import sys

for _p in ("/opt/trn_rl_repo",):
    if _p not in sys.path:
        sys.path.append(_p)

import numpy as np
import ml_dtypes
from contextlib import ExitStack

from concourse import bass, bacc, mybir, tile
from concourse import bass_utils

P = 128
NCORES = 8
BPB = 7  # dest blocks per gather batch
P_DROP = 0.5

_dt = mybir.dt
_op = mybir.AluOpType


def _preprocess(rows, cols, vals, N):
    """Sort/pad edges into per-core gather + selector streams."""
    E = rows.shape[0]
    rows = np.asarray(rows, dtype=np.int64)
    cols = np.asarray(cols, dtype=np.int64)
    vals = np.asarray(vals, dtype=np.float32)

    RPC = -(-N // (NCORES * P)) * P  # rows per core, multiple of 128
    NB = RPC // P                    # dest blocks per core
    split = min(-(-N // 2), 32000)
    assert split < 32768 and (N - split) < 32768

    core = rows // RPC
    binc = (rows % RPC) // P
    r_rel = (rows % P).astype(np.int64)
    half = (cols >= split).astype(np.int64)
    idx16 = np.where(half == 1, cols - split, cols).astype(np.int16)

    group = (core * NB + binc) * 2 + half
    order = np.argsort(group, kind="stable")
    g_sorted = group[order]

    gcounts = np.bincount(group, minlength=NCORES * NB * 2)
    gstarts = np.concatenate([[0], np.cumsum(gcounts)])[:-1]
    rank = np.arange(E, dtype=np.int64) - gstarts[g_sorted]

    # chunks per (block, half): max over cores, at least 1
    counts = gcounts.reshape(NCORES, NB, 2)
    k = np.maximum(1, -(-counts.max(axis=0) // P))  # [NB, 2]

    # slot layout per core: per batch of BPB blocks: all A halves, then all B
    batches = [list(range(i, min(i + BPB, NB))) for i in range(0, NB, BPB)]
    group_order = []
    for bl in batches:
        group_order += [(b, 0) for b in bl] + [(b, 1) for b in bl]
    sizes = np.array([k[b, h] * P for (b, h) in group_order], dtype=np.int64)
    offs = np.concatenate([[0], np.cumsum(sizes)])
    T_slots = int(offs[-1])
    T_chunks = T_slots // P
    slot_off = np.zeros((NB, 2), dtype=np.int64)
    for gi, (b, h) in enumerate(group_order):
        slot_off[b, h] = offs[gi]

    bh_sorted = g_sorted % (NB * 2)
    c_sorted = g_sorted // (NB * 2)
    slot = slot_off.reshape(-1)[bh_sorted] + rank
    assert (rank < k.reshape(-1)[bh_sorted] * P).all()

    idx_pad = np.zeros((NCORES, T_slots), np.int16)
    v_pad = np.zeros((NCORES, T_slots), np.float32)
    r_pad = np.zeros((NCORES, T_slots), np.int64)
    idx_pad[c_sorted, slot] = idx16[order]
    v_pad[c_sorted, slot] = vals[order]
    r_pad[c_sorted, slot] = r_rel[order]

    # idx element k lives at [k%16, k//16]; the Q7 SWDGE cores each read
    # their own 16-partition group, so replicate into all 8 groups.
    idx_w = np.zeros((NCORES, 128, T_slots // 16), np.int16)
    wrapped = idx_pad.reshape(NCORES, T_slots // 16, 16).transpose(0, 2, 1)
    for g in range(8):
        idx_w[:, g * 16:(g + 1) * 16, :] = wrapped

    # Host-built selector stream: S[core][p, t*128 + d] = vals * (dest_rel==d)
    # for slot t*128+p.  DMA'd per batch; rhs of the SpMM matmuls.
    bf = ml_dtypes.bfloat16
    s_all = np.zeros((NCORES, T_slots, P), bf)
    slot_idx = np.arange(T_slots)
    for c in range(NCORES):
        s_all[c, slot_idx, r_pad[c]] = v_pad[c].astype(bf)
    # pad slots have v=0: row stays ~zero (single 0 write at d=0 harmless)
    s_w = np.ascontiguousarray(
        s_all.reshape(NCORES, T_chunks, P, P).transpose(0, 2, 1, 3)
        .reshape(NCORES, P, T_chunks * P))

    return dict(
        RPC=RPC, NB=NB, split=split, k=k, batches=batches,
        slot_off=slot_off, T_slots=T_slots, T_chunks=T_chunks,
        idx_w=idx_w, s_w=s_w,
    )


def _build(N, meta):
    """Build the (per-core identical) Tile program."""
    NB = meta["NB"]
    RPC = meta["RPC"]
    split = meta["split"]
    k = meta["k"]
    batches = meta["batches"]
    slot_off = meta["slot_off"]
    T_slots = meta["T_slots"]
    T_chunks = meta["T_chunks"]

    nc = bacc.Bacc("TRN2", target_bir_lowering=False, debug=False,
                   num_swdge_queues=4)
    xb = nc.dram_tensor("xb", [N, P], _dt.bfloat16, kind="ExternalInput").ap()
    wt = nc.dram_tensor("wt", [P, P], _dt.bfloat16, kind="ExternalInput").ap()
    bt = nc.dram_tensor("bt", [1, P], _dt.bfloat16, kind="ExternalInput").ap()
    on = nc.dram_tensor("on", [1, P], _dt.bfloat16, kind="ExternalInput").ap()
    ix = nc.dram_tensor("ix", [128, T_slots // 16], _dt.int16,
                        kind="ExternalInput").ap()
    ss = nc.dram_tensor("ss", [128, T_chunks * P], _dt.bfloat16,
                        kind="ExternalInput").ap()
    du = nc.dram_tensor("du", [RPC, P], _dt.float32, kind="ExternalInput").ap()
    out = nc.dram_tensor("out", [RPC, P], _dt.float32,
                         kind="ExternalOutput").ap()

    with tile.TileContext(nc) as tc, ExitStack() as ctx:
        const = ctx.enter_context(tc.tile_pool(name="const", bufs=1))
        g_pool = ctx.enter_context(tc.tile_pool(name="g", bufs=3))
        s_pool = ctx.enter_context(tc.tile_pool(name="s", bufs=2))
        ix_pool = ctx.enter_context(tc.tile_pool(name="ix", bufs=2))
        h_pool = ctx.enter_context(tc.tile_pool(name="h", bufs=3))
        du_pool = ctx.enter_context(tc.tile_pool(name="du", bufs=2))
        ep_pool = ctx.enter_context(tc.tile_pool(name="ep", bufs=4))
        o_pool = ctx.enter_context(tc.tile_pool(name="o", bufs=2))
        psum_h = ctx.enter_context(tc.tile_pool(name="ph", bufs=2, space="PSUM"))
        psum_o = ctx.enter_context(tc.tile_pool(name="po", bufs=2, space="PSUM"))

        w_t = const.tile([P, P], _dt.bfloat16)
        nc.sync.dma_start(w_t[:], wt)
        b_t = const.tile([1, P], _dt.bfloat16)
        nc.sync.dma_start(b_t[:], bt)
        ones_t = const.tile([1, P], _dt.bfloat16)
        nc.sync.dma_start(ones_t[:], on)

        for bi, blocks in enumerate(batches):
            nblk = len(blocks)
            cA = int(sum(k[b, 0] for b in blocks))
            cB = int(sum(k[b, 1] for b in blocks))
            ctot = cA + cB
            base_chunk = int(slot_off[blocks[0], 0]) // P
            base_slot = base_chunk * P

            G = g_pool.tile([P, ctot * P], _dt.bfloat16, tag="G")
            S = s_pool.tile([P, ctot * P], _dt.bfloat16, tag="S")
            nc.sync.dma_start(
                S[:], ss[:, base_chunk * P: (base_chunk + ctot) * P])
            ixA = ix_pool.tile([128, cA * 8], _dt.int16, tag="ixA")
            nc.sync.dma_start(
                ixA[:], ix[:, base_slot // 16: base_slot // 16 + cA * 8])
            ixB = ix_pool.tile([128, cB * 8], _dt.int16, tag="ixB")
            bslotB = base_slot + cA * P
            nc.sync.dma_start(
                ixB[:], ix[:, bslotB // 16: bslotB // 16 + cB * 8])

            gA = G[:, 0: cA * P].rearrange("p (c e) -> p c e", e=P)
            nc.gpsimd.dma_gather(
                out_ap=gA, in_ap=xb[0:split, :], idxs_ap=ixA[:],
                num_idxs=cA * P, num_idxs_reg=cA * P, elem_size=P,
                single_packet=False, queue_num=(2 * bi) % 4)
            gB = G[:, cA * P: ctot * P].rearrange("p (c e) -> p c e", e=P)
            nc.gpsimd.dma_gather(
                out_ap=gB, in_ap=xb[split:N, :], idxs_ap=ixB[:],
                num_idxs=cB * P, num_idxs_reg=cB * P, elem_size=P,
                single_packet=False, queue_num=(2 * bi + 1) % 4)

            dut = du_pool.tile([P, nblk * P], _dt.float32, tag="du")
            r0 = blocks[0] * P
            nc.sync.dma_start(
                dut[:].rearrange("p (b d) -> p b d", d=P),
                du[r0: r0 + nblk * P, :].rearrange("(b p) d -> p b d", p=P))
            ot = o_pool.tile([P, nblk * P], _dt.float32, tag="ot")

            for j, b in enumerate(blocks):
                chunks = []
                for h in (0, 1):
                    c0 = int(slot_off[b, h]) // P - base_chunk
                    chunks += list(range(c0, c0 + int(k[b, h])))
                Hp = psum_h.tile([P, P], _dt.float32)
                for i, lc in enumerate(chunks):
                    nc.tensor.matmul(
                        out=Hp[:], lhsT=G[:, lc * P: (lc + 1) * P],
                        rhs=S[:, lc * P: (lc + 1) * P],
                        start=(i == 0), stop=(i == len(chunks) - 1))
                Hs = h_pool.tile([P, P], _dt.bfloat16, tag="Hs")
                nc.scalar.copy(Hs[:], Hp[:])
                Op = psum_o.tile([P, P], _dt.float32)
                nc.tensor.matmul(Op[:], lhsT=ones_t[:], rhs=b_t[:],
                                 start=True, stop=False)
                nc.tensor.matmul(Op[:], lhsT=Hs[:], rhs=w_t[:],
                                 start=False, stop=True)
                m2 = ep_pool.tile([P, P], _dt.float32, tag="m2")
                nc.vector.tensor_scalar(
                    out=m2[:], in0=dut[:, j * P:(j + 1) * P],
                    scalar1=float(P_DROP), scalar2=1.0 / (1.0 - P_DROP),
                    op0=_op.is_ge, op1=_op.mult)
                nc.vector.scalar_tensor_tensor(
                    out=ot[:, j * P:(j + 1) * P], in0=Op[:], scalar=0.0,
                    in1=m2[:], op0=_op.max, op1=_op.mult)
            nc.sync.dma_start(
                out[r0: r0 + nblk * P, :].rearrange("(b p) d -> p b d", p=P),
                ot[:].rearrange("p (b d) -> p b d", d=P))

    nc.compile()
    return nc


def _make_in_maps(X, W, b, drop_u, meta):
    N = X.shape[0]
    RPC = meta["RPC"]
    bf = ml_dtypes.bfloat16
    xb = np.ascontiguousarray(X.astype(bf))
    wt = np.ascontiguousarray(W.astype(bf))
    bt = np.ascontiguousarray(b.reshape(1, P).astype(bf))
    on = np.ones((1, P), dtype=bf)
    du_pad = np.ones((NCORES * RPC, P), np.float32)
    du_pad[:N] = drop_u
    in_maps = []
    for c in range(NCORES):
        in_maps.append(dict(
            xb=xb, wt=wt, bt=bt, on=on,
            ix=meta["idx_w"][c], ss=meta["s_w"][c],
            du=np.ascontiguousarray(du_pad[c * RPC: (c + 1) * RPC]),
        ))
    return in_maps


def kernel(rows, cols, vals, X, W, b, drop_u):
    N = X.shape[0]
    assert X.shape[1] == P and W.shape == (P, P)
    meta = _preprocess(rows, cols, vals, N)
    nc = _build(N, meta)
    in_maps = _make_in_maps(
        np.asarray(X, np.float32), np.asarray(W, np.float32),
        np.asarray(b, np.float32), np.asarray(drop_u, np.float32), meta)
    res = bass_utils.run_bass_kernel_spmd(
        nc, in_maps, core_ids=list(range(NCORES)))
    out = np.concatenate([res.results[c]["out"] for c in range(NCORES)], axis=0)
    return out[:N].astype(np.float32)


# revision 7
# speedup vs baseline: 2.5719x; 1.5265x over previous
"""GCN layer (SpMM + dense + dropout/relu) on 8 Trainium2 NeuronCores.

Strategy (v2)
-------------
Destination-node sharding: core c owns output rows [c*RPC, (c+1)*RPC).
Edges are partitioned by destination owner on the host, sorted by
(dest-block, source-half), padded to 128-edge chunks.

Per core, per 128-edge chunk:
  - G[128 edges, 128 dims] = X_bf16[cols] via batched SWDGE dma_gather,
    round-robined over all 4 SWDGE queues (4 Q7 core-pairs generate
    descriptors in parallel; ~2.7 ns/edge wall).
  - S[128 edges, 128 dest] = vals * onehot(dest_rel): HOST-built bf16
    stream, DMA'd in per batch (kills the per-chunk DVE one-hot build,
    which dominates when built on-device: ~600ns fixed per DVE op).
  - H_T[dims, dest-block] += G.T @ S   (TensorE, PSUM accumulate)
Per 128-row dest block:
  - OUT = H @ W + b   (bias seeded by a rank-1 ones@b matmul into PSUM)
  - out = relu(OUT) * ((drop_u >= 0.5) * 2)   (2 DVE ops)
drop_u loads and out stores are batched per 7-block gather batch.

No collectives: each core gathers from its own replica of X.
dma_gather indices are int16, so X is split in two < 32768-row tables and
each (block, half) edge run is padded to a 128 multiple.
"""

import sys

for _p in ("/opt/trn_rl_repo",):
    if _p not in sys.path:
        sys.path.append(_p)

import numpy as np
import ml_dtypes
from contextlib import ExitStack

from concourse import bass, bacc, mybir, tile
from concourse import bass_utils

P = 128
NCORES = 8
BPB = 7  # dest blocks per gather batch
P_DROP = 0.5

_dt = mybir.dt
_op = mybir.AluOpType


def _preprocess(rows, cols, vals, N):
    """Sort/pad edges into per-core gather + selector streams."""
    E = rows.shape[0]
    rows = np.asarray(rows, dtype=np.int64)
    cols = np.asarray(cols, dtype=np.int64)
    vals = np.asarray(vals, dtype=np.float32)

    RPC = -(-N // (NCORES * P)) * P  # rows per core, multiple of 128
    NB = RPC // P                    # dest blocks per core
    split = min(-(-N // 2), 32000)
    assert split < 32768 and (N - split) < 32768

    core = rows // RPC
    binc = (rows % RPC) // P
    r_rel = (rows % P).astype(np.int64)
    half = (cols >= split).astype(np.int64)
    idx16 = np.where(half == 1, cols - split, cols).astype(np.int16)

    group = (core * NB + binc) * 2 + half
    order = np.argsort(group, kind="stable")
    g_sorted = group[order]

    gcounts = np.bincount(group, minlength=NCORES * NB * 2)
    gstarts = np.concatenate([[0], np.cumsum(gcounts)])[:-1]
    rank = np.arange(E, dtype=np.int64) - gstarts[g_sorted]

    # chunks per (block, half): max over cores, at least 1
    counts = gcounts.reshape(NCORES, NB, 2)
    k = np.maximum(1, -(-counts.max(axis=0) // P))  # [NB, 2]

    # slot layout per core: per batch of BPB blocks: all A halves, then all B
    batches = [list(range(i, min(i + BPB, NB))) for i in range(0, NB, BPB)]
    group_order = []
    for bl in batches:
        group_order += [(b, 0) for b in bl] + [(b, 1) for b in bl]
    sizes = np.array([k[b, h] * P for (b, h) in group_order], dtype=np.int64)
    offs = np.concatenate([[0], np.cumsum(sizes)])
    T_slots = int(offs[-1])
    T_chunks = T_slots // P
    slot_off = np.zeros((NB, 2), dtype=np.int64)
    for gi, (b, h) in enumerate(group_order):
        slot_off[b, h] = offs[gi]

    bh_sorted = g_sorted % (NB * 2)
    c_sorted = g_sorted // (NB * 2)
    slot = slot_off.reshape(-1)[bh_sorted] + rank
    assert (rank < k.reshape(-1)[bh_sorted] * P).all()

    idx_pad = np.zeros((NCORES, T_slots), np.int16)
    v_pad = np.zeros((NCORES, T_slots), np.float32)
    r_pad = np.zeros((NCORES, T_slots), np.int64)
    idx_pad[c_sorted, slot] = idx16[order]
    v_pad[c_sorted, slot] = vals[order]
    r_pad[c_sorted, slot] = r_rel[order]

    # idx element k lives at [k%16, k//16]; the Q7 SWDGE cores each read
    # their own 16-partition group, so replicate into all 8 groups.
    idx_w = np.zeros((NCORES, 128, T_slots // 16), np.int16)
    wrapped = idx_pad.reshape(NCORES, T_slots // 16, 16).transpose(0, 2, 1)
    for g in range(8):
        idx_w[:, g * 16:(g + 1) * 16, :] = wrapped

    # Host-built selector stream: S[core][p, t*128 + d] = vals * (dest_rel==d)
    # for slot t*128+p.  DMA'd per batch; rhs of the SpMM matmuls.
    bf = ml_dtypes.bfloat16
    s_all = np.zeros((NCORES, T_slots, P), bf)
    slot_idx = np.arange(T_slots)
    for c in range(NCORES):
        s_all[c, slot_idx, r_pad[c]] = v_pad[c].astype(bf)
    # pad slots have v=0: row stays ~zero (single 0 write at d=0 harmless)
    s_w = np.ascontiguousarray(
        s_all.reshape(NCORES, T_chunks, P, P).transpose(0, 2, 1, 3)
        .reshape(NCORES, P, T_chunks * P))

    return dict(
        RPC=RPC, NB=NB, split=split, k=k, batches=batches,
        slot_off=slot_off, T_slots=T_slots, T_chunks=T_chunks,
        idx_w=idx_w, s_w=s_w,
    )


def _build(N, meta):
    """Build the (per-core identical) Tile program."""
    NB = meta["NB"]
    RPC = meta["RPC"]
    split = meta["split"]
    k = meta["k"]
    batches = meta["batches"]
    slot_off = meta["slot_off"]
    T_slots = meta["T_slots"]
    T_chunks = meta["T_chunks"]

    _build._q = 0
    nc = bacc.Bacc("TRN2", target_bir_lowering=False, debug=False,
                   num_swdge_queues=4)
    xb = nc.dram_tensor("xb", [N, P], _dt.bfloat16, kind="ExternalInput").ap()
    wt = nc.dram_tensor("wt", [P, P], _dt.bfloat16, kind="ExternalInput").ap()
    bt = nc.dram_tensor("bt", [1, P], _dt.bfloat16, kind="ExternalInput").ap()
    on = nc.dram_tensor("on", [1, P], _dt.bfloat16, kind="ExternalInput").ap()
    ix = nc.dram_tensor("ix", [128, T_slots // 16], _dt.int16,
                        kind="ExternalInput").ap()
    ss = nc.dram_tensor("ss", [128, T_chunks * P], _dt.bfloat16,
                        kind="ExternalInput").ap()
    du = nc.dram_tensor("du", [RPC, P], _dt.float32, kind="ExternalInput").ap()
    out = nc.dram_tensor("out", [RPC, P], _dt.float32,
                         kind="ExternalOutput").ap()

    with tile.TileContext(nc) as tc, ExitStack() as ctx:
        const = ctx.enter_context(tc.tile_pool(name="const", bufs=1))
        g_pool = ctx.enter_context(tc.tile_pool(name="g", bufs=3))
        s_pool = ctx.enter_context(tc.tile_pool(name="s", bufs=2))
        ix_pool = ctx.enter_context(tc.tile_pool(name="ix", bufs=2))
        h_pool = ctx.enter_context(tc.tile_pool(name="h", bufs=3))
        du_pool = ctx.enter_context(tc.tile_pool(name="du", bufs=2))
        ep_pool = ctx.enter_context(tc.tile_pool(name="ep", bufs=4))
        o_pool = ctx.enter_context(tc.tile_pool(name="o", bufs=2))
        psum_h = ctx.enter_context(tc.tile_pool(name="ph", bufs=2, space="PSUM"))
        psum_o = ctx.enter_context(tc.tile_pool(name="po", bufs=2, space="PSUM"))

        w_t = const.tile([P, P], _dt.bfloat16)
        nc.sync.dma_start(w_t[:], wt)
        b_t = const.tile([1, P], _dt.bfloat16)
        nc.sync.dma_start(b_t[:], bt)
        ones_t = const.tile([1, P], _dt.bfloat16)
        nc.sync.dma_start(ones_t[:], on)

        for bi, blocks in enumerate(batches):
            nblk = len(blocks)
            cA = int(sum(k[b, 0] for b in blocks))
            cB = int(sum(k[b, 1] for b in blocks))
            ctot = cA + cB
            base_chunk = int(slot_off[blocks[0], 0]) // P
            base_slot = base_chunk * P

            G = g_pool.tile([P, ctot * P], _dt.bfloat16, tag="G")
            S = s_pool.tile([P, ctot * P], _dt.bfloat16, tag="S")
            nc.scalar.dma_start(
                S[:], ss[:, base_chunk * P: (base_chunk + ctot) * P])
            ixA = ix_pool.tile([128, cA * 8], _dt.int16, tag="ixA")
            nc.sync.dma_start(
                ixA[:], ix[:, base_slot // 16: base_slot // 16 + cA * 8])
            ixB = ix_pool.tile([128, cB * 8], _dt.int16, tag="ixB")
            bslotB = base_slot + cA * P
            nc.sync.dma_start(
                ixB[:], ix[:, bslotB // 16: bslotB // 16 + cB * 8])

            for (coff, ccnt, src_ap, ixt) in (
                    (0, cA, xb[0:split, :], ixA),
                    (cA, cB, xb[split:N, :], ixB)):
                done = 0
                while done < ccnt:
                    cc = min(32, ccnt - done)
                    gpart = G[:, (coff + done) * P: (coff + done + cc) * P] \
                        .rearrange("p (c e) -> p c e", e=P)
                    nc.gpsimd.dma_gather(
                        out_ap=gpart, in_ap=src_ap,
                        idxs_ap=ixt[:, done * 8: (done + cc) * 8],
                        num_idxs=cc * P, num_idxs_reg=cc * P, elem_size=P,
                        single_packet=False, queue_num=_build._q % 4)
                    _build._q += 1
                    done += cc

            dut = du_pool.tile([P, nblk * P], _dt.float32, tag="du")
            r0 = blocks[0] * P
            nc.sync.dma_start(
                dut[:].rearrange("p (b d) -> p b d", d=P),
                du[r0: r0 + nblk * P, :].rearrange("(b p) d -> p b d", p=P))
            ot = o_pool.tile([P, nblk * P], _dt.float32, tag="ot")

            for j, b in enumerate(blocks):
                chunks = []
                for h in (0, 1):
                    c0 = int(slot_off[b, h]) // P - base_chunk
                    chunks += list(range(c0, c0 + int(k[b, h])))
                Hp = psum_h.tile([P, P], _dt.float32)
                for i, lc in enumerate(chunks):
                    nc.tensor.matmul(
                        out=Hp[:], lhsT=G[:, lc * P: (lc + 1) * P],
                        rhs=S[:, lc * P: (lc + 1) * P],
                        start=(i == 0), stop=(i == len(chunks) - 1))
                Hs = h_pool.tile([P, P], _dt.bfloat16, tag="Hs")
                nc.scalar.copy(Hs[:], Hp[:])
                Op = psum_o.tile([P, P], _dt.float32)
                nc.tensor.matmul(Op[:], lhsT=ones_t[:], rhs=b_t[:],
                                 start=True, stop=False)
                nc.tensor.matmul(Op[:], lhsT=Hs[:], rhs=w_t[:],
                                 start=False, stop=True)
                m2 = ep_pool.tile([P, P], _dt.float32, tag="m2")
                nc.vector.tensor_scalar(
                    out=m2[:], in0=dut[:, j * P:(j + 1) * P],
                    scalar1=float(P_DROP), scalar2=1.0 / (1.0 - P_DROP),
                    op0=_op.is_ge, op1=_op.mult)
                nc.vector.scalar_tensor_tensor(
                    out=ot[:, j * P:(j + 1) * P], in0=Op[:], scalar=0.0,
                    in1=m2[:], op0=_op.max, op1=_op.mult)
            nc.sync.dma_start(
                out[r0: r0 + nblk * P, :].rearrange("(b p) d -> p b d", p=P),
                ot[:].rearrange("p (b d) -> p b d", d=P))

    nc.compile()
    return nc


def _make_in_maps(X, W, b, drop_u, meta):
    N = X.shape[0]
    RPC = meta["RPC"]
    bf = ml_dtypes.bfloat16
    xb = np.ascontiguousarray(X.astype(bf))
    wt = np.ascontiguousarray(W.astype(bf))
    bt = np.ascontiguousarray(b.reshape(1, P).astype(bf))
    on = np.ones((1, P), dtype=bf)
    du_pad = np.ones((NCORES * RPC, P), np.float32)
    du_pad[:N] = drop_u
    in_maps = []
    for c in range(NCORES):
        in_maps.append(dict(
            xb=xb, wt=wt, bt=bt, on=on,
            ix=meta["idx_w"][c], ss=meta["s_w"][c],
            du=np.ascontiguousarray(du_pad[c * RPC: (c + 1) * RPC]),
        ))
    return in_maps


def kernel(rows, cols, vals, X, W, b, drop_u):
    N = X.shape[0]
    assert X.shape[1] == P and W.shape == (P, P)
    meta = _preprocess(rows, cols, vals, N)
    nc = _build(N, meta)
    in_maps = _make_in_maps(
        np.asarray(X, np.float32), np.asarray(W, np.float32),
        np.asarray(b, np.float32), np.asarray(drop_u, np.float32), meta)
    res = bass_utils.run_bass_kernel_spmd(
        nc, in_maps, core_ids=list(range(NCORES)))
    out = np.concatenate([res.results[c]["out"] for c in range(NCORES)], axis=0)
    return out[:N].astype(np.float32)
